# revision 3
# baseline (speedup 1.0000x reference)
"""Trainium2 Bass kernel v6 for nn_ChemModel (DMPNN-style message-passing GNN).

Self-contained: call kernel(**inputs) with the full (unsharded) inputs from
setup_inputs(); returns the full [N_GRAPHS, 1] float32 output.

v3 = v2 plus:
  * one batched is_equal per gather instruction builds ALL one-hots
    (chunk columns + shifted spill columns appended per instruction)
  * 4 block accumulators packed per PSUM bank ([P,512] group tiles),
    one wide DVE add per (range, group) close instead of per block
  * h state, pooling, and FFN kept f32 for accuracy
"""
import math
import numpy as np

import concourse.bass as bass
from concourse import bacc
import concourse.mybir as mybir
import concourse.tile as tile
from concourse.bass_utils import run_bass_kernel_spmd
from concourse import library_config

P = 128
NCORES = 8
GIDX_N = 2048              # indices per dma_gather instruction
GCH = GIDX_N // P          # chunks per gather instruction (16)
F32 = mybir.dt.float32
BF16 = mybir.dt.bfloat16
I16 = mybir.dt.int16
MSG_DT = BF16
GB = 4                     # blocks per PSUM accumulator group


def _relu():
    return mybir.ActivationFunctionType.Relu


def _wrap_idx16(flat):
    """[n] int array -> [16, n//16] int16 wrapped layout (replicated to 128
    partitions on device at load time)."""
    n = flat.shape[0]
    assert n % 16 == 0
    return flat.reshape(n // 16, 16).T.astype(np.int16)


class _Plan:
    pass


# ----------------------------------------------------------------------------
# canonical chunk scheduling (identical across cores; per-core content)
# ----------------------------------------------------------------------------

def _sched_range(cnts, NB):
    """cnts: [NCORES, NB] per-core per-block edge counts for one range."""
    need = np.maximum(cnts.max(axis=0), 1).astype(np.int64)
    while True:
        chunks_blo = []
        grp = [[] for _ in range(NB)]
        slack = 0
        for b in range(NB):
            use = min(slack, int(need[b]))
            rem = int(need[b]) - use
            mb = (rem + P - 1) // P
            for _ in range(mb):
                grp[b].append(len(chunks_blo))
                chunks_blo.append(b)
            slack = (mb * P - rem) if mb > 0 else 0
        spill_from = [None] * NB
        for b in range(1, NB):
            if grp[b - 1]:
                spill_from[b] = grp[b - 1][-1]
        bump = None
        for k in range(NCORES):
            free = [P] * len(chunks_blo)
            for b in range(NB):
                cnt = int(cnts[k, b])
                sf = spill_from[b]
                if sf is not None:
                    take = min(free[sf], cnt)
                    free[sf] -= take
                    cnt -= take
                for t in grp[b]:
                    take = min(free[t], cnt)
                    free[t] -= take
                    cnt -= take
                if cnt > 0:
                    bump = b
                    break
            if bump is not None:
                break
        if bump is None:
            return chunks_blo, grp, spill_from
        need[bump] += P


def _assign_edges(by_b, chunks_blo, grp, spill_from, NB):
    slots = [[] for _ in chunks_blo]
    free = [P] * len(chunks_blo)
    for b in range(NB):
        es = by_b.get(b, [])
        i = 0
        sf = spill_from[b]
        if sf is not None and i < len(es):
            take = min(free[sf], len(es) - i)
            slots[sf].extend(es[i:i + take])
            free[sf] -= take
            i += take
        for t in grp[b]:
            if i >= len(es):
                break
            take = min(free[t], len(es) - i)
            slots[t].extend(es[i:i + take])
            free[t] -= take
            i += take
        assert i == len(es), "canonical schedule infeasible"
    return slots


def _sched_ops(chunks_blo, grp, NB, Tp):
    """Canonical schedule for one range.

    Returns dict:
      n_instr, instr_ncols[i], instr_spills[i] (chunk list),
      ops: ordered list of (instr, col_j, t, b, start, stop, g_close)
        col_j indexes the per-instruction one-hot tile columns:
        chunk t -> col t%16; spill of chunk t -> col 16+rank.
      g_close: group index to close after this op, or None.
    """
    n_instr = Tp // GCH
    raw = []
    for t, b_lo in enumerate(chunks_blo):
        raw.append((t, 0, b_lo))
        if grp[b_lo] and t == grp[b_lo][-1] and b_lo + 1 < NB:
            raw.append((t, 1, b_lo + 1))
    first_b, last_b = {}, {}
    glast = {}
    for oi, (t, w, b) in enumerate(raw):
        first_b.setdefault(b, oi)
        last_b[b] = oi
        glast[b // GB] = oi
    instr_spills = [[] for _ in range(n_instr)]
    for (t, w, b) in raw:
        if w == 1:
            instr_spills[t // GCH].append(t)
    instr_ncols = [GCH + len(s) for s in instr_spills]
    ops = []
    for oi, (t, w, b) in enumerate(raw):
        i = t // GCH
        if w == 0:
            col_j = t % GCH
        else:
            col_j = GCH + instr_spills[i].index(t)
        gc = b // GB if glast[b // GB] == oi else None
        ops.append((i, col_j, t, b, oi == first_b[b], oi == last_b[b], gc))
    return {"n_instr": n_instr, "instr_ncols": instr_ncols,
            "instr_spills": instr_spills, "ops": ops}


def _emit_dloc(dloc_col_list, sched):
    """dloc_col_list: per-chunk [P] f32 arrays (len Tp). Emits columns in
    device layout: per instr, 16 chunk cols then shifted spill cols.
    Returns [P, ncols_total] f32."""
    cols = []
    for i in range(sched["n_instr"]):
        for t in range(i * GCH, (i + 1) * GCH):
            cols.append(dloc_col_list[t])
        for t in sched["instr_spills"][i]:
            cols.append(dloc_col_list[t] - P)
    import ml_dtypes
    return np.ascontiguousarray(
        np.stack(cols, axis=1).astype(ml_dtypes.bfloat16))


# ----------------------------------------------------------------------------
# host-side planning
# ----------------------------------------------------------------------------

def _host_prep(x, edge_index, edge_attr, batch, depth, G):
    N, E = x.shape[0], edge_index.shape[1]
    H = 128
    src = edge_index[0].astype(np.int64)
    dst = edge_index[1].astype(np.int64)
    batch = batch.astype(np.int64)

    assert N % NCORES == 0
    NSH = N // NCORES
    NB = math.ceil(NSH / P)
    SHP = NB * P
    TBL = NCORES * SHP
    RNG = 2 * SHP
    NRANGE = math.ceil(TBL / RNG)
    assert RNG <= 32512 and NRANGE == 4

    def rowof(v):
        return (v // NSH) * SHP + (v % NSH)

    row_src = rowof(src)
    ZROW = NSH  # zero row, local within any range window

    plan = _Plan()
    plan.N, plan.E, plan.H, plan.G = N, E, H, G
    plan.NSH, plan.NB, plan.SHP, plan.TBL, plan.RNG = NSH, NB, SHP, TBL, RNG
    plan.depth = int(depth)
    plan.GW = min(512, G)

    core_of = dst // NSH
    dloc_all = dst - core_of * NSH

    def percore_by_block(sel):
        """Returns cnts [NCORES, NB] and per-core dict b->edge list
        (dloc-sorted)."""
        cnts = np.zeros((NCORES, NB), np.int64)
        percore = []
        for k in range(NCORES):
            selk = sel & (core_of == k)
            eidx = np.where(selk)[0]
            dl = dloc_all[eidx]
            order = np.argsort(dl, kind="stable")
            eidx = eidx[order]
            dl = dl[order]
            cnts[k] = np.bincount(dl // P, minlength=NB)
            by_b = {}
            pos = 0
            for b in range(NB):
                c = int(cnts[k, b])
                if c:
                    by_b[b] = eidx[pos:pos + c]
                pos += c
            percore.append(by_b)
        return cnts, percore

    def build_gather_pass(edge_sel, key_rows):
        pa = {"ranges": []}
        idx_cols = [[] for _ in range(NCORES)]
        dloc_tabs = [[] for _ in range(NCORES)]
        for r in range(NRANGE):
            sel = edge_sel & (key_rows // RNG == r)
            cnts, percore = percore_by_block(sel)
            chunks_blo, grp, spill_from = _sched_range(cnts, NB)
            T = len(chunks_blo)
            Tp = ((T + GCH - 1) // GCH) * GCH
            sched = _sched_ops(chunks_blo, grp, NB, Tp)
            sched["Tp"] = Tp
            pa["ranges"].append(sched)
            for k in range(NCORES):
                slots = _assign_edges(percore[k], chunks_blo, grp,
                                      spill_from, NB)
                idx = np.full(Tp * P, ZROW, np.int64)
                dloc_list = []
                for t in range(Tp):
                    dlc = np.full(P, -1.0, np.float32)
                    if t < T and slots[t]:
                        es = np.asarray(slots[t], np.int64)
                        idx[t * P:t * P + len(es)] = key_rows[es] - r * RNG
                        dlc[:len(es)] = dloc_all[es] - chunks_blo[t] * P
                    dloc_list.append(dlc)
                idx_cols[k].append(idx)
                dloc_tabs[k].append(_emit_dloc(dloc_list, sched))
        pa["n_instr_total"] = sum(rr["n_instr"] for rr in pa["ranges"])
        pa["ncols_total"] = sum(sum(rr["instr_ncols"])
                                for rr in pa["ranges"])

        def wrap_all(cols):
            out = []
            for c in cols:
                for ii in range(len(c) // GIDX_N):
                    out.append(_wrap_idx16(c[ii * GIDX_N:(ii + 1) * GIDX_N]))
            return np.concatenate(out, axis=1)

        pa["percore_idx"] = [wrap_all(idx_cols[k]) for k in range(NCORES)]
        pa["percore_dloc"] = [
            np.concatenate(dloc_tabs[k], axis=1) for k in range(NCORES)]
        return pa

    # loop pass: all edges, keyed by row_src (same tables reused per iter)
    plan.loop = build_gather_pass(np.ones(E, bool), row_src)
    # final stream a: edge ids < N, keyed by rowof(edge id)
    eid = np.arange(E)
    row_eid = rowof(np.minimum(eid, N - 1))
    plan.fina = build_gather_pass(eid < N, row_eid)

    # final stream b: edge ids >= N, no gather; canonical single-range sched
    cnts, percore_b = percore_by_block(eid >= N)
    chunks_blo, grp, spill_from = _sched_range(cnts, NB)
    Tb = len(chunks_blo)
    Tbp = ((Tb + GCH - 1) // GCH) * GCH
    schedb = _sched_ops(chunks_blo, grp, NB, Tbp)
    schedb["Tp"] = Tbp
    plan.finb = schedb
    plan.finb_ncols = sum(schedb["instr_ncols"])

    import ml_dtypes
    xea_finb = []
    dloc_finb = []
    for k in range(NCORES):
        slots = _assign_edges(percore_b[k], chunks_blo, grp, spill_from, NB)
        xea = np.zeros((Tbp * P, 7), np.float32)
        dloc_list = []
        for t in range(Tbp):
            dlc = np.full(P, -1.0, np.float32)
            if t < Tb and slots[t]:
                es = np.asarray(slots[t], np.int64)
                xea[t * P:t * P + len(es), :4] = x[src[es]]
                xea[t * P:t * P + len(es), 4:] = edge_attr[es]
                dlc[:len(es)] = dloc_all[es] - chunks_blo[t] * P
            dloc_list.append(dlc)
        xea_finb.append(np.ascontiguousarray(
            xea.T.astype(ml_dtypes.bfloat16)))
        dloc_finb.append(_emit_dloc(dloc_list, schedb))

    # init xea7 per slot (slot s of core k = edge id k*NSH+s)
    xea_init = []
    for k in range(NCORES):
        e0 = k * NSH
        xea = np.zeros((SHP, 7), np.float32)
        xea[:NSH, :4] = x[src[e0:e0 + NSH]]
        xea[:NSH, 4:] = edge_attr[e0:e0 + NSH]
        xea_init.append(np.ascontiguousarray(xea.T.astype(ml_dtypes.bfloat16)))

    # node features per core (for W_a stage)
    xT_cores = []
    for k in range(NCORES):
        xs = np.zeros((SHP, x.shape[1]), np.float32)
        xs[:NSH] = x[k * NSH:(k + 1) * NSH]
        xT_cores.append(np.ascontiguousarray(xs.T))

    # pooling bases
    plan.g_bases = []
    batchloc = []
    for k in range(NCORES):
        gb = int(batch[k * NSH])
        ge = int(batch[(k + 1) * NSH - 1])
        assert ge - gb < plan.GW, f"graph span {ge - gb} >= {plan.GW}"
        plan.g_bases.append(gb)
        bl = np.full((SHP,), -1.0, np.float32)
        bl[:NSH] = batch[k * NSH:(k + 1) * NSH] - gb
        batchloc.append(np.ascontiguousarray(bl.reshape(NB, P).T))

    plan.percore = []
    for k in range(NCORES):
        plan.percore.append({
            "gaL": plan.loop["percore_idx"][k],
            "dlocL": plan.loop["percore_dloc"][k],
            "gaF": plan.fina["percore_idx"][k],
            "dlocFa": plan.fina["percore_dloc"][k],
            "dlocFb": dloc_finb[k],
            "xeaF": xea_finb[k],
            "xeaI": xea_init[k],
            "xT": xT_cores[k],
            "batchloc": batchloc[k],
        })
    return plan


# ----------------------------------------------------------------------------
# device kernel
# ----------------------------------------------------------------------------

def _build(plan, split=True):
    H, NB, SHP, TBL, RNG = plan.H, plan.NB, plan.SHP, plan.TBL, plan.RNG
    G, GW = plan.G, plan.GW
    depth = plan.depth
    NRANGE = 4
    NGRP = (NB + GB - 1) // GB

    MAXNC = max(
        max(max(rr["instr_ncols"]) for rr in plan.loop["ranges"]),
        max(max(rr["instr_ncols"]) for rr in plan.fina["ranges"]),
        max(plan.finb["instr_ncols"]))

    nc = bacc.Bacc(num_devices=NCORES)

    def din(name, shape, dt=F32):
        return nc.declare_dram_parameter(name, list(shape), dt, isOutput=False)

    WiTb = din("WiTb", [7, H], BF16)
    WmT = din("WmT", [H, H])
    WaxT = din("WaxT", [4, H])
    WahT = din("WahT", [H, H])
    W1T = din("W1T", [H, 4 * H])
    W2T = din("W2T", [4 * H, H])
    WlastT = din("WlastT", [H, 1])
    b1r = din("b1r", [H, 4])
    b2r = din("b2r", [H, 1])
    blast = din("blast", [1, 1])
    iotaLO = din("iotaLO", [P, P], BF16)
    iotaG = din("iotaG", [P, GW])
    ident = din("ident", [P, P])

    nIL = plan.loop["n_instr_total"]
    nCL = plan.loop["ncols_total"]
    nIFa = plan.fina["n_instr_total"]
    nCFa = plan.fina["ncols_total"]
    TFb = plan.finb["Tp"]
    nCFb = plan.finb_ncols
    gaL_in = din("gaL", [16, nIL * (GIDX_N // 16)], I16)
    dlocL_in = din("dlocL", [P, nCL], BF16)
    gaF_in = din("gaF", [16, nIFa * (GIDX_N // 16)], I16)
    dlocFa_in = din("dlocFa", [P, nCFa], BF16)
    dlocFb_in = din("dlocFb", [P, nCFb], BF16)
    xeaF_in = din("xeaF", [7, TFb * P], BF16)
    xeaI_in = din("xeaI", [7, SHP], BF16)
    xT_in = din("xT", [4, SHP])
    batchloc_in = din("batchloc", [P, NB])

    out_ext = nc.declare_dram_parameter("out", [G, 1], F32, isOutput=True)

    RG = list(range(NCORES))

    with tile.TileContext(nc) as tc:
        nc.gpsimd.load_library(library_config.mlp)
        with (
            tc.tile_pool(name="cp", bufs=1) as cp,
            tc.tile_pool(name="sb", bufs=3) as sb,
            tc.tile_pool(name="ps", bufs=2, space="PSUM") as ps,
            tc.tile_pool(name="dr", bufs=1, space="DRAM") as dr,
        ):
            def cload(name, srct):
                tl = cp.tile([srct.shape[0], srct.shape[1]], srct.dtype,
                             name=name)
                nc.sync.dma_start(out=tl[:], in_=srct[:, :])
                return tl

            WiTb_s = cload("WiTb_s", WiTb)
            WmT_s = cload("WmT_s", WmT)
            WaxT_s = cload("WaxT_s", WaxT)
            WahT_s = cload("WahT_s", WahT)
            W1T_s = cload("W1T_s", W1T)
            W2T_f = []
            for f in range(4):
                tl = cp.tile([P, H], F32, name=f"W2T_{f}")
                nc.sync.dma_start(out=tl[:], in_=W2T[f * P:(f + 1) * P, :])
                W2T_f.append(tl)
            WlastT_s = cload("WlastT_s", WlastT)
            b1r_s = cload("b1r_s", b1r)
            b2r_s = cload("b2r_s", b2r)
            blast_s = cload("blast_s", blast)
            iotaLO_s = cload("iotaLO_s", iotaLO)
            iotaG_s = cload("iotaG_s", iotaG)
            ident_s = cload("ident_s", ident)
            def cload_rep16(name, srct):
                tl = cp.tile([P, srct.shape[1]], srct.dtype, name=name)
                for a in range(8):
                    nc.sync.dma_start(out=tl[a * 16:(a + 1) * 16, :],
                                      in_=srct[:, :])
                return tl

            gaL_s = cload_rep16("gaL_s", gaL_in)
            dlocL_s = cload("dlocL_s", dlocL_in)
            gaF_s = cload_rep16("gaF_s", gaF_in)
            dlocFa_s = cload("dlocFa_s", dlocFa_in)
            dlocFb_s = cload("dlocFb_s", dlocFb_in)
            batchloc_s = cload("batchloc_s", batchloc_in)

            hA_T = cp.tile([P, SHP], F32, name="hA_T")

            mA_locals = [dr.tile([SHP, H], MSG_DT, name=f"mA_local{i}")
                         for i in range(2)]
            mA_tbls = [dr.tile([TBL, H], MSG_DT, name=f"mA_tbl{it}",
                               addr_space="Shared") for it in range(depth)]
            hfin_local = dr.tile([SHP, H], MSG_DT, name="hfin_local")
            hfin_tbl = dr.tile([TBL, H], MSG_DT, name="hfin_tbl",
                               addr_space="Shared")
            gwin_local = dr.tile([P, GW], F32, name="gwin_local")
            gwin_all = dr.tile([NCORES * P, GW], F32, name="gwin_all",
                               addr_space="Shared")

            def allgather(local, table):
                nc.gpsimd.collective_compute(
                    "AllGather", mybir.AluOpType.bypass,
                    replica_groups=[RG], ins=[local[:]], outs=[table[:]])

            def gather(dst_ap, table_ap, idx_sb, inst_col):
                nc.gpsimd.dma_gather(
                    out_ap=dst_ap, in_ap=table_ap,
                    idxs_ap=idx_sb[:, inst_col * (GIDX_N // 16):
                                   (inst_col + 1) * (GIDX_N // 16)],
                    num_idxs=GIDX_N, num_idxs_reg=GIDX_N,
                    elem_size=H, single_packet=False)

            def iota3d(iota_s, n):
                return bass.AP(iota_s.tensor, iota_s.offset,
                               [list(iota_s.ap[0]), [0, n],
                                list(iota_s.ap[1])])

            # ---------------- init: hA_T = relu(W_i . xea7) ----------------
            NSTR = (SHP + 511) // 512
            for j in range(NSTR):
                s0 = j * 512
                w = min(512, SHP - s0)
                xs = sb.tile([7, 512], BF16, name="xeaIs", tag="xea", bufs=2)
                nc.sync.dma_start(out=xs[:, :w], in_=xeaI_in[:, s0:s0 + w])
                pi = ps.tile([P, 512], F32, name="pinit", tag="pbig",
                             space="PSUM")
                nc.tensor.matmul(out=pi[:, :w], lhsT=WiTb_s[:],
                                 rhs=xs[:, :w], start=True, stop=True)
                nc.scalar.activation(out=hA_T[:, s0:s0 + w], in_=pi[:, :w],
                                     func=_relu())

            # scatter-accumulate machinery shared by all passes.
            # state_close(g, acc_ap, width_cols): fold group acc into state.
            def run_pass(sched, dloc_s, dloc_c0, state_close, make_lhsT):
                """sched: one range's schedule. make_lhsT(i) -> tile holding
                the 16 chunks' [P, GCH, H] edge rows for instr i."""
                ops_by_instr = [[] for _ in range(sched["n_instr"])]
                for op in sched["ops"]:
                    ops_by_instr[op[0]].append(op)
                gaccs = {}
                ccol = dloc_c0
                for i in range(sched["n_instr"]):
                    ncols = sched["instr_ncols"][i]
                    lhsT = make_lhsT(i)
                    oh = sb.tile([P, MAXNC, P], MSG_DT, name="oh",
                                 tag="oh0", bufs=3)
                    dsl = dloc_s[:, ccol:ccol + ncols]
                    nc.vector.tensor_tensor(
                        out=oh[:, :ncols, :],
                        in0=dsl.to_broadcast([P, ncols, P]),
                        in1=iota3d(iotaLO_s, ncols),
                        op=mybir.AluOpType.is_equal)
                    for (_, col_j, t, b, st, sp, gc) in ops_by_instr[i]:
                        g = b // GB
                        if g not in gaccs:
                            gaccs[g] = ps.tile([P, GB * H], F32, name="gacc",
                                               tag="pacc4", bufs=2,
                                               space="PSUM")
                        c = t % GCH
                        bo = (b % GB) * P
                        nc.tensor.matmul(out=gaccs[g][:, bo:bo + P],
                                         lhsT=lhsT[:, c, :],
                                         rhs=oh[:, col_j, :],
                                         start=st, stop=sp)
                        if gc is not None:
                            wcols = min(GB * P, (NB - gc * GB) * P)
                            state_close(gc, gaccs.pop(gc), wcols)
                    ccol += ncols
                assert not gaccs
                return ccol

            def gather_pass(pranges, table, idx_s, dloc_s, state_close):
                icol = 0
                ccol = 0
                for r, sched in enumerate(pranges):
                    base_icol = icol

                    def mk(i, _r=r, _b=base_icol):
                        gt = sb.tile([P, GCH, H], MSG_DT, name="gt",
                                     tag="gat", bufs=3)
                        gather(gt[:], table[_r * RNG:(_r + 1) * RNG, :],
                               idx_s, _b + i)
                        return gt

                    ccol = run_pass(sched, dloc_s, ccol, state_close, mk)
                    icol += sched["n_instr"]

            # ---------------- message-passing iterations ----------------
            # mA for the NEXT iteration (or hfin transposes after the last)
            # is emitted once a group has closed in all 4 ranges, hiding the
            # staging inside the scatter burst. v6 per-range accumulation is
            # unchanged.
            def emit_stage_group(g, last_iter, dst_local):
                j0 = g * GB
                nb4 = min(GB, NB - j0)
                stg = sb.tile([P, 4, H], MSG_DT, name="mstg", tag="mstg",
                              bufs=2)
                if not last_iter:
                    pm = ps.tile([P, 4, H], F32, name="pm", tag="pbig",
                                 space="PSUM")
                    for jj in range(nb4):
                        b = j0 + jj
                        nc.tensor.matmul(out=pm[:, jj, :],
                                         lhsT=hA_T[:, b * P:(b + 1) * P],
                                         rhs=WmT_s[:], start=True, stop=True)
                    nc.scalar.activation(out=stg[:, :nb4, :],
                                         in_=pm[:, :nb4, :], func=_relu())
                    nc.sync.dma_start(
                        out=dst_local[j0 * P:(j0 + nb4) * P, :].rearrange(
                            "(c p) h -> p c h", p=P),
                        in_=stg[:, :nb4, :])
                else:
                    for jj in range(nb4):
                        b = j0 + jj
                        pt = ps.tile([P, H], F32, name="ptf", tag="pacc",
                                     bufs=2, space="PSUM")
                        nc.tensor.transpose(out=pt[:],
                                            in_=hA_T[:, b * P:(b + 1) * P],
                                            identity=ident_s[:])
                        nc.vector.tensor_copy(out=stg[:, jj, :], in_=pt[:])
                    nc.sync.dma_start(
                        out=hfin_local[j0 * P:(j0 + nb4) * P, :].rearrange(
                            "(c p) h -> p c h", p=P),
                        in_=stg[:, :nb4, :])

            for g0 in range(NGRP):  # iteration-0 mA from init state
                emit_stage_group(g0, False, mA_locals[0])

            for it in range(depth):
                mA_tbl = mA_tbls[it]
                allgather(mA_locals[it % 2], mA_tbl)
                last_iter = (it == depth - 1)
                nxt_local = mA_locals[(it + 1) % 2]
                ncl = {}
                pending = []

                def close_loop(g, acc, wcols, _ncl=ncl, _pending=pending,
                               _last=last_iter, _nl=nxt_local):
                    nc.vector.tensor_add(
                        out=hA_T[:, g * GB * P:g * GB * P + wcols],
                        in0=hA_T[:, g * GB * P:g * GB * P + wcols],
                        in1=acc[:, :wcols])
                    _ncl[g] = _ncl.get(g, 0) + 1
                    if _ncl[g] == 4:
                        _pending.append(g)
                        if len(_pending) > 1:
                            emit_stage_group(_pending.pop(0), _last, _nl)

                gather_pass(plan.loop["ranges"], mA_tbl, gaL_s, dlocL_s,
                            close_loop)
                for g in pending:
                    emit_stage_group(g, last_iter, nxt_local)

            # ---------------- final aggregation ----------------
            allgather(hfin_local, hfin_tbl)

            first_close = set()

            def close_fin(g, acc, wcols):
                if g in first_close:
                    nc.vector.tensor_add(
                        out=hA_T[:, g * GB * P:g * GB * P + wcols],
                        in0=hA_T[:, g * GB * P:g * GB * P + wcols],
                        in1=acc[:, :wcols])
                else:
                    first_close.add(g)
                    nc.vector.tensor_copy(
                        out=hA_T[:, g * GB * P:g * GB * P + wcols],
                        in_=acc[:, :wcols])

            # stream b: recompute h0 from xea7 columns; overlaps AllGather
            def mk_finb(i):
                xs = sb.tile([7, GIDX_N], BF16, name="xeaFs", tag="xea",
                             bufs=2)
                nc.sync.dma_start(
                    out=xs[:], in_=xeaF_in[:, i * GIDX_N:(i + 1) * GIDX_N])
                ot = sb.tile([P, GCH, H], MSG_DT, name="h0t", tag="gat",
                             bufs=3)
                for c4 in range(4):
                    ph = ps.tile([P, 4, H], F32, name="ph0", tag="pbig",
                                 space="PSUM")
                    for cc in range(4):
                        c = c4 * 4 + cc
                        nc.tensor.matmul(out=ph[:, cc, :],
                                         lhsT=xs[:, c * P:(c + 1) * P],
                                         rhs=WiTb_s[:], start=True, stop=True)
                    nc.scalar.activation(out=ot[:, c4 * 4:(c4 + 1) * 4, :],
                                         in_=ph[:], func=_relu())
                return ot

            run_pass(plan.finb, dlocFb_s, 0, close_fin, mk_finb)

            # stream a: gather final h rows by edge id
            gather_pass(plan.fina["ranges"], hfin_tbl, gaF_s, dlocFa_s,
                        close_fin)

            # node embedding -> relu(x.Wax + ne.Wah) -> pooling
            gps = ps.tile([P, GW], F32, name="gps", tag="gps", space="PSUM",
                          bufs=1)
            for j0 in range(0, NB, 8):
                nb8 = min(8, NB - j0)
                xts = sb.tile([4, 8 * P], F32, name="xts", tag="xts", bufs=2)
                nc.sync.dma_start(out=xts[:, :nb8 * P],
                                  in_=xT_in[:, j0 * P:(j0 + nb8) * P])
                for jj in range(nb8):
                    b = j0 + jj
                    p2 = ps.tile([P, H], F32, name="p2f", tag="pacc",
                                 bufs=2, space="PSUM")
                    nc.tensor.matmul(out=p2[:],
                                     lhsT=hA_T[:, b * P:(b + 1) * P],
                                     rhs=WahT_s[:], start=True, stop=False)
                    nc.tensor.matmul(out=p2[:],
                                     lhsT=xts[:, jj * P:(jj + 1) * P],
                                     rhs=WaxT_s[:], start=False, stop=True)
                    ne2 = sb.tile([P, H], F32, name="ne2", tag="ne2",
                                  bufs=3)
                    nc.scalar.activation(out=ne2[:], in_=p2[:], func=_relu())
                    ohg = sb.tile([P, GW], F32, name="ohg", tag="ohg",
                                  bufs=3)
                    nc.vector.tensor_tensor(
                        out=ohg[:],
                        in0=batchloc_s[:, b:b + 1].to_broadcast([P, GW]),
                        in1=iotaG_s[:], op=mybir.AluOpType.is_equal)
                    nc.tensor.matmul(out=gps[:], lhsT=ne2[:], rhs=ohg[:],
                                     start=(b == 0), stop=(b == NB - 1))

            tgw = sb.tile([P, GW], F32, name="tgw", tag="tgw", bufs=2)
            nc.vector.tensor_copy(out=tgw[:], in_=gps[:])
            nc.sync.dma_start(out=gwin_local[:, :], in_=tgw[:])
            allgather(gwin_local, gwin_all)
            gfull = cp.tile([P, G], F32, name="gfull")
            nc.vector.memset(gfull[:], 0.0)
            for j in range(NCORES):
                wj = min(GW, G - plan.g_bases[j])
                tw = sb.tile([P, GW], F32, name="twj", tag="tgw", bufs=2)
                nc.sync.dma_start(out=tw[:],
                                  in_=gwin_all[j * P:(j + 1) * P, :])
                nc.vector.tensor_add(
                    out=gfull[:, plan.g_bases[j]:plan.g_bases[j] + wj],
                    in0=gfull[:, plan.g_bases[j]:plan.g_bases[j] + wj],
                    in1=tw[:, :wj])

            # ---------------- FFN (replicated on all cores) ----------------
            NGC = math.ceil(G / 512)
            z2sb = cp.tile([P, G], F32, name="z2sb")
            nc.vector.memset(z2sb[:], 0.0)
            for f in range(4):
                z1f = sb.tile([P, G], F32, name="z1f", tag="z1f", bufs=2)
                for gc in range(NGC):
                    g0, g1 = gc * 512, min((gc + 1) * 512, G)
                    pz = ps.tile([P, 512], F32, name="pz", tag="pbig",
                                 space="PSUM")
                    nc.tensor.matmul(out=pz[:, :g1 - g0],
                                     lhsT=W1T_s[:, f * P:(f + 1) * P],
                                     rhs=gfull[:, g0:g1], start=True,
                                     stop=True)
                    nc.scalar.activation(out=z1f[:, g0:g1],
                                         in_=pz[:, :g1 - g0],
                                         func=_relu(), bias=b1r_s[:, f:f + 1])
                for gc in range(NGC):
                    g0, g1 = gc * 512, min((gc + 1) * 512, G)
                    pz2 = ps.tile([P, 512], F32, name="pz2", tag="pbig",
                                  space="PSUM")
                    nc.tensor.matmul(out=pz2[:, :g1 - g0], lhsT=W2T_f[f][:],
                                     rhs=z1f[:, g0:g1], start=True, stop=True)
                    nc.vector.tensor_add(out=z2sb[:, g0:g1],
                                         in0=z2sb[:, g0:g1],
                                         in1=pz2[:, :g1 - g0])
            nc.vector.tensor_add(out=z2sb[:], in0=z2sb[:],
                                 in1=b2r_s[:, 0:1].to_broadcast([P, G]))
            orow = sb.tile([1, G], F32, name="orow", tag="orow", bufs=2)
            for gc in range(NGC):
                g0, g1 = gc * 512, min((gc + 1) * 512, G)
                po = ps.tile([1, 512], F32, name="po", tag="pbig",
                             space="PSUM")
                nc.tensor.matmul(out=po[:, :g1 - g0], lhsT=WlastT_s[:],
                                 rhs=z2sb[:, g0:g1], start=True, stop=True)
                nc.vector.tensor_add(
                    out=orow[:, g0:g1], in0=po[:, :g1 - g0],
                    in1=blast_s[0:1, 0:1].to_broadcast([1, g1 - g0]))
            nc.sync.dma_start(out=out_ext[:, :], in_=orow[:])

    nc.compile()
    if split:
        _split_excess_waits(nc)
    return nc


def _split_excess_waits(nc, max_waits=1):
    k = 0
    for f in nc.m.functions:
        for bb in f.blocks:
            new = []
            for ins in bb.instructions:
                si = ins.sync_info
                if si is not None and len(si.on_wait) > max_waits:
                    waits = list(si.on_wait)
                    for w in waits[:-max_waits]:
                        nop = mybir.InstNoOp(name=f"I-waitsplit-{k}",
                                             engine=ins.engine)
                        k += 1
                        nop.sync_info = mybir.SyncInfo(on_wait=[w],
                                                       on_update=[])
                        new.append(nop)
                    si.on_wait = waits[-max_waits:]
                new.append(ins)
            bb.instructions = new
    return k


# ----------------------------------------------------------------------------
# inputs
# ----------------------------------------------------------------------------

def _in_maps(plan, weights):
    import ml_dtypes
    H = plan.H
    GW = plan.GW
    bf = ml_dtypes.bfloat16
    com = {
        "WiTb": np.ascontiguousarray(weights["W_i"].T.astype(bf)),
        "WmT": np.ascontiguousarray(weights["W_m"].T),
        "WaxT": np.ascontiguousarray(weights["W_a"][:, :4].T),
        "WahT": np.ascontiguousarray(weights["W_a"][:, 4:].T),
        "W1T": np.ascontiguousarray(weights["W1"].T),
        "W2T": np.ascontiguousarray(weights["W2"].T),
        "WlastT": np.ascontiguousarray(weights["W_last"].T),
        "b1r": np.ascontiguousarray(weights["b1"].reshape(4, H).T),
        "b2r": weights["b2"].reshape(H, 1).copy(),
        "blast": weights["b_last"].reshape(1, 1).copy(),
        "iotaLO": np.tile(np.arange(P, dtype=np.float32), (P, 1)).astype(bf),
        "iotaG": np.tile(np.arange(GW, dtype=np.float32), (P, 1)),
        "ident": np.eye(P, dtype=np.float32),
    }
    maps = []
    for k in range(NCORES):
        m = dict(com)
        m.update(plan.percore[k])
        maps.append(m)
    return maps


def _prep_all(x, edge_index, edge_attr, batch, depth, weights, G):
    plan = _host_prep(np.asarray(x, np.float32), np.asarray(edge_index),
                      np.asarray(edge_attr, np.float32), np.asarray(batch),
                      int(depth), G)
    maps = _in_maps(plan, weights)
    return plan, maps


def kernel(x, edge_index, edge_attr, batch, depth,
           W_i, W_m, W_a, W1, b1, W2, b2, W_last, b_last):
    weights = {
        "W_i": np.asarray(W_i, np.float32), "W_m": np.asarray(W_m, np.float32),
        "W_a": np.asarray(W_a, np.float32), "W1": np.asarray(W1, np.float32),
        "b1": np.asarray(b1, np.float32), "W2": np.asarray(W2, np.float32),
        "b2": np.asarray(b2, np.float32),
        "W_last": np.asarray(W_last, np.float32),
        "b_last": np.asarray(b_last, np.float32),
    }
    G = 2048
    plan, maps = _prep_all(x, edge_index, edge_attr, batch, depth, weights, G)
    nc = _build(plan, split=True)
    res = run_bass_kernel_spmd(nc, maps, list(range(NCORES)))
    return np.asarray(res.results[0]["out"]).reshape(G, 1).astype(np.float32)
